# revision 9
# baseline (speedup 1.0000x reference)
"""Haar 3D wavelet transform (2x2x2 stride-2 conv, 8 sign filters) on 8 trn2 cores.

Input  x: (2, 3, 33, 512, 512) f32, w: (8, 1, 2, 2, 2) f32.
Output:   (2, 24, 17, 256, 256) f32.

The kernel is pure streaming (every input element used once), so HW time is
bound by HBM traffic.  Two levers vs the f32 baseline:
  1. All transfers are fp16 (tolerance 2e-2; measured rel err ~7e-4), halving
     HBM bytes.  Host converts f32->fp16 while packing and back on unpack.
  2. The whole 2x2x2 butterfly runs on TensorE: the host packs all three
     offset bits (dt, dh, dw) into the partition index, so ONE stationary
     128x128 matmul computes all 8 filters; DVE/ACT only evacuate PSUM.

Work unit: quarter-unit qu = ((b, c, t), s), s = vertical quarter of a frame
pair.  408 qus / 8 cores = exactly 51 each, no padding.  Device layout per qu:
    partition p = dt*64 + dh*32 + dw*16 + g   (g in [0,16))
    free      f = r*256 + wo                  (r in [0,4))
    input  (dt, h = 128 s + g*8 + r*2 + dh, w = 2 wo + dw)
    output row k*16+g, col r*256+wo  ->  (filter k, ho = 64 s + g*4 + r, wo)
Each core's 51 qus are packed partition-major into one (128, 52224) fp16
array, streamed in 1 MiB chunks: DMA-in (SP queue) -> 8 matmuls/chunk ->
PSUM evacuation+cast alternating DVE/ACT -> DMA-out (ACT queue).
"""

import numpy as np

N_CORES = 8
B, C, T_IN, H, W = 2, 3, 33, 512, 512
T_OUT = 17
QU_PER_CORE = 51            # 2*3*17*4 / 8
NCOL = QU_PER_CORE * 1024   # 52224 free columns per core
CHUNKS = [1024, 2048, 4096] + [8192] * 5 + [3072, 1024]
assert sum(CHUNKS) == NCOL


def _build_nc(legalize=True):
    import concourse.bass as bass
    import concourse.mybir as mybir
    from concourse.tile import TileContext

    f16 = mybir.dt.float16
    nc = bass.Bass()
    xin = nc.declare_dram_parameter("xin", [128, NCOL], f16, isOutput=False)
    wmat = nc.declare_dram_parameter("wmat", [128, 128], f16, isOutput=False)
    yout = nc.declare_dram_parameter("yout", [128, NCOL], f16, isOutput=True)

    with TileContext(nc) as tc:
        with (
            tc.tile_pool(name="const", bufs=1) as cpool,
            tc.tile_pool(name="xpool", bufs=4) as xpool,
            tc.tile_pool(name="rpool", bufs=4) as rpool,
            tc.tile_pool(name="ppool", bufs=2, space="PSUM") as ppool,
        ):
            wt = cpool.tile([128, 128], f16)
            nc.sync.dma_start(out=wt[:], in_=wmat[:])

            # ~28 dummy matmuls (~4us of PE busy) flip the HAM clock gate to
            # 2.4 GHz before the first real chunk arrives, halving the first
            # chunks' matmul latency and pulling the first output DMA earlier.
            pwarm = ppool.tile([128, 2048], mybir.dt.float32, tag="p")
            for i in range(28):
                nc.tensor.matmul(
                    pwarm[:, (i % 4) * 128:(i % 4) * 128 + 128],
                    lhsT=wt[:], rhs=wt[:], start=True, stop=True)

            c0 = 0
            evac = 0
            for ci, ch in enumerate(CHUNKS):
                xt = xpool.tile([128, 8192], f16, tag="x")
                nc.sync.dma_start(out=xt[:, :ch], in_=xin[:, c0:c0 + ch])
                rt = rpool.tile([128, 8192], f16, tag="r")
                for off in range(0, ch, 2048):
                    blk = min(2048, ch - off)
                    pt = ppool.tile([128, 2048], mybir.dt.float32, tag="p")
                    for m in range(0, blk, 512):
                        nc.tensor.matmul(
                            pt[:, m:m + 512],
                            lhsT=wt[:],
                            rhs=xt[:, off + m:off + m + 512],
                            start=True, stop=True)
                    # PSUM f32 -> SBUF fp16 (cast); alternate engines.
                    if evac % 2 == 0:
                        nc.vector.tensor_copy(rt[:, off:off + blk], pt[:, :blk])
                    else:
                        nc.scalar.copy(rt[:, off:off + blk], pt[:, :blk])
                    evac += 1
                # Alternate the output between the ACT HWDGE ring and the
                # gpsimd SWDGE ring: each ring's transfers run FIFO behind a
                # wait on that chunk's evacuation, so two rings let one drain
                # while the other waits.  The last chunk rides the SP ring,
                # free once all inputs are issued, for a three-ring tail.
                if ci == len(CHUNKS) - 1:
                    nc.sync.dma_start(out=yout[:, c0:c0 + ch], in_=rt[:, :ch])
                elif ci % 2 == 0:
                    nc.scalar.dma_start(out=yout[:, c0:c0 + ch], in_=rt[:, :ch])
                else:
                    nc.gpsimd.dma_start(out=yout[:, c0:c0 + ch], in_=rt[:, :ch])
                c0 += ch

    if legalize:
        _legalize_waits(nc)
    return nc


def _legalize_waits(nc, limit=1):
    """walrus codegen rejects instructions carrying more than ~1 sem wait
    (e.g. Matmult's LoadWeights slot).  Move excess waits onto NoOp
    instructions inserted just before the instruction on the same engine
    queue -- semantically identical (all waits still precede execution)."""
    import bass_rust

    fn = nc.m.functions[0]
    lastblk = fn.blocks[-1]
    eng_ns = {
        "PE": nc.tensor, "DVE": nc.vector, "Activation": nc.scalar,
        "SP": nc.sync, "Pool": nc.gpsimd,
    }
    # NoOp codegen requires >=1 sem update. Give each engine its own dummy
    # sem (ids picked from the top of the 150..255 HW range, skipping any id
    # already referenced) so no counting or cross-proc rule is disturbed.
    used_ids = set()
    for blk in fn.blocks:
        for inst in blk.instructions:
            si = getattr(inst, "sync_info", None)
            if si is None:
                continue
            for w in si.on_wait:
                used_ids.add(w.id)
            for upd in si.on_update:
                used_ids.add(upd.id)
    avail = [i for i in range(255, 149, -1) if i not in used_ids]
    eng_upd = {}
    for k, en in enumerate(["PE", "DVE", "Activation", "SP", "Pool"]):
        eng_upd[en] = bass_rust.SyncUpdate(
            sync_type="semaphore", id=avail[k], ant_name=f"waitnop_{en}",
            update_mode="sem-inc", update_value=1, update_reg=None)

    def copy_wait(w):
        return bass_rust.SyncWait(
            sync_type=w.sync_type, id=w.id, ant_name=w.ant_name,
            wait_mode=w.wait_mode, wait_value=w.wait_value, wait_reg=w.wait_reg)

    def make_nop(engine_name, waits):
        ns = eng_ns[engine_name]
        ns.nop(hint="waitcarrier")
        nop = lastblk.instructions.pop()
        raw = getattr(nop, "inst", nop)
        raw.sync_info = bass_rust.SyncInfo(
            on_wait=[copy_wait(w) for w in waits],
            on_update=[eng_upd[engine_name]])
        return raw

    for blk in fn.blocks:
        insts = blk.instructions
        i = 0
        while i < len(insts):
            inst = insts[i]
            ty = type(inst).__name__
            si = getattr(inst, "sync_info", None)
            if (ty not in ("InstEventSemaphore", "InstNoOp")
                    and si is not None and len(si.on_wait) > limit):
                ename = str(inst.engine).split(".")[-1]
                waits = [copy_wait(w) for w in si.on_wait]
                upds = list(si.on_update)
                extra, keep = waits[:-limit], waits[-limit:]
                for w in extra:
                    insts.insert(i, make_nop(ename, [w]))
                    i += 1
                inst.sync_info = bass_rust.SyncInfo(
                    on_wait=keep, on_update=upds)
            i += 1


def _make_wmat(w):
    """128x128 stationary butterfly matrix: W[(dt,dh,dw,g), (k,g)] = w[k,dt,dh,dw].
    Works for ANY 8-filter 2x2x2 kernel (no separability assumption)."""
    w8 = np.asarray(w, dtype=np.float32).reshape(8, 2, 2, 2)
    wm = np.zeros((128, 128), dtype=np.float32)
    g = np.arange(16)
    for k in range(8):
        for dt in range(2):
            for dh in range(2):
                for dw in range(2):
                    wm[dt * 64 + dh * 32 + dw * 16 + g, k * 16 + g] = w8[k, dt, dh, dw]
    return wm.astype(np.float16)


def _pack_input(x):
    """x (2,3,33,512,512) f32 -> list of 8 (128, NCOL) fp16 per-core arrays."""
    x16 = x.astype(np.float16)
    pairs = np.empty((T_OUT, 2), dtype=np.int64)
    for t in range(T_OUT):
        pairs[t, 0] = max(2 * t - 1, 0)
        pairs[t, 1] = 2 * t
    full = x16[:, :, pairs]                       # (b, c, t, dt, 512, 512)
    # h = s*128 + g*8 + r*2 + dh ; w = wo*2 + dw
    arr = full.reshape(B, C, T_OUT, 2, 4, 16, 4, 2, 256, 2)
    #                  b  c  t     dt s  g   r  dh wo  dw
    arr = arr.transpose(0, 1, 2, 4, 3, 7, 9, 5, 6, 8)
    # (b, c, t, s, dt, dh, dw, g, r, wo)
    arr = np.ascontiguousarray(arr).reshape(8 * QU_PER_CORE, 128, 1024)
    return [
        np.ascontiguousarray(
            arr[QU_PER_CORE * m:QU_PER_CORE * (m + 1)].transpose(1, 0, 2)
        ).reshape(128, NCOL)
        for m in range(N_CORES)
    ]


def _unpack_output(youts):
    """list of 8 (128, NCOL) fp16 -> (2, 24, 17, 256, 256) f32."""
    Y = np.stack(youts)                           # (8, 128, NCOL)
    arr = Y.reshape(N_CORES, 128, QU_PER_CORE, 1024).transpose(0, 2, 1, 3)
    arr = arr.reshape(B, C, T_OUT, 4, 8, 16, 4, 256)
    #                 b  c  t     s  k  g   r  wo
    arr = arr.transpose(0, 4, 1, 2, 3, 5, 6, 7)
    # (b, k, c, t, s, g, r, wo): channel = k*3+c, ho = s*64 + g*4 + r
    return np.ascontiguousarray(arr).reshape(
        B, 24, T_OUT, 256, 256).astype(np.float32)


LAST_RESULT = None


def kernel(x, w):
    import os
    from concourse.bass_utils import run_bass_kernel_spmd

    x = np.asarray(x, dtype=np.float32)
    wm = _make_wmat(w)
    in_maps = [{"xin": xc, "wmat": wm} for xc in _pack_input(x)]

    nc = _build_nc()
    kw = {}
    if os.environ.get("KERNEL_PROFILE") == "1":
        kw = dict(trace=True, tmpdir=os.environ.get("KERNEL_PROFILE_DIR"))
    res = run_bass_kernel_spmd(nc, in_maps, core_ids=list(range(N_CORES)), **kw)
    global LAST_RESULT
    LAST_RESULT = res

    return _unpack_output([np.asarray(res.results[m]["yout"]) for m in range(N_CORES)])


if __name__ == "__main__":
    rng = np.random.default_rng(0)
    x = rng.standard_normal((B, C, T_IN, H, W), dtype=np.float32)
    SCALE = 0.3536
    flags = np.array([[0, 0, 0], [0, 0, 1], [0, 1, 0], [0, 1, 1],
                      [1, 0, 0], [1, 0, 1], [1, 1, 0], [1, 1, 1]])
    t, h, ww = np.meshgrid(np.arange(2), np.arange(2), np.arange(2), indexing="ij")
    sign = (-1.0) ** (flags[:, 0, None, None, None] * t
                      + flags[:, 1, None, None, None] * h
                      + flags[:, 2, None, None, None] * ww)
    wf = (SCALE * sign).reshape(8, 1, 2, 2, 2).astype(np.float32)
    y = kernel(x, wf)
    print(y.shape, y.dtype)
